# revision 18
# baseline (speedup 1.0000x reference)
"""GAT layer (nn_GAT_layer_67619965108552) as a Trainium2 Bass/Tile SPMD kernel.

Structure exploited (validated vs reference to 5.2e-3 in bf16):
  With n=8192, the buggy-but-faithful pair indexing collapses:
    rows i < 4096:  scores[i, j] = u[2i + (j >= 4096)],  u = x @ (W@a1 + W@a2)
    rows i >= 4096: scores[i, j] = tt[j mod 4096],  tt[q] = s1[2q] + s2[2q+1]
  After leaky_relu + adj masking + softmax, attn @ out reduces to masked
  row-sum matmuls against f-scaled out:
    ysb[h][mb] = sum_{kc in half h} adjT(mb,kc)^T @ (f * [out | 1])[kc]
    res = sigmoid((al1*ysbL + al2*ysbR)[:, :256] / (...)[:, 256])
  Top-half cores: f = 1, al1/al2 = exp(lrelu(u at even/odd rows)); bottom-half
  cores: f = exp(lrelu(tt)), al = 1. One instruction stream for all cores;
  divergence is data-driven (g flag / select masks).

Single fused pipeline in 8 super-tiles: x^T (bf16) streams on the HWDGE queue
while the per-core transposed adj slice (int32 -> bf16 cast) streams on the
SWDGE queue in 4MB column-group DMAs with 4KB-contiguous runs — no on-chip
transposes at all; u/s1/s2 come free as 3 extra matmul columns; per-pair
score vectors are extracted with constant 0/1 parity-pick matmuls; f-scaling
is per-chunk so nothing serializes globally.

Sharding: rows of adj / output across 8 cores (1024 each); x/weight/att_vec
replicated (each core computes the full out = x@W as the rhs of its matmuls).
"""
import ml_dtypes
import numpy as np
from contextlib import ExitStack

import concourse.bass as bass
import concourse.tile as tile
from concourse import bacc, mybir
from concourse.bass_utils import run_bass_kernel_spmd

F32 = mybir.dt.float32
BF16 = mybir.dt.bfloat16
I32 = mybir.dt.int32

N = 8192           # nodes
FIN = 512          # input features
FOUT = 256         # output features
P = 128
NB = N // P        # 64 row-blocks of out
NCORES = 8
RPC = N // NCORES  # 1024 rows per core
MB = RPC // P      # 8 output row-blocks per core
NST = 8            # super-tiles (adj column groups of 1024)


def build_program():
    nc = bacc.Bacc("TRN2", target_bir_lowering=False, debug=False,
                   num_devices=NCORES)

    xt_d = nc.dram_tensor("xt", [FIN, N], BF16, kind="ExternalInput")
    w_d = nc.dram_tensor("w", [FIN, FOUT], F32, kind="ExternalInput")
    attb_d = nc.dram_tensor("attb", [P, 2 * FOUT], F32, kind="ExternalInput")
    # per-core transposed adj slice: adjt[j, r] = adj[c*RPC + r, j]
    adjt_d = nc.dram_tensor("adjt", [N, RPC], I32, kind="ExternalInput")
    # gcol[:, 0] = g (1.0 top-half cores, 0.0 bottom), gcol[:, 1] = 1-g
    g_d = nc.dram_tensor("gcol", [P, 2], F32, kind="ExternalInput")
    # selg[p, mb, m] = g * (m == 8c + mb) : per-core pair select for alphas
    selg_d = nc.dram_tensor("selg", [P, MB, 32], F32, kind="ExternalInput")
    # parity-pick matrices [E0 | E1 | Eo0 | Eo1]
    emat_d = nc.dram_tensor("emat", [P, 4, P], BF16, kind="ExternalInput")
    y_d = nc.dram_tensor("y", [RPC, FOUT], F32, kind="ExternalOutput")

    Exp = mybir.ActivationFunctionType.Exp
    Sigmoid = mybir.ActivationFunctionType.Sigmoid
    AX = mybir.AxisListType.X
    ADD = mybir.AluOpType.add
    MULT = mybir.AluOpType.mult

    with tile.TileContext(nc) as tc, ExitStack() as ctx:
        constp = ctx.enter_context(tc.tile_pool(name="const", bufs=1))

        wtile = constp.tile([P, 4, FOUT], F32)
        attb = constp.tile([P, 2 * FOUT], F32)

        # persistent tensors
        wrhs = constp.tile([P, 4, FOUT + 3], BF16)   # [W | wu | w1 | w2]
        outb = constp.tile([P, NB, FOUT + 1], BF16)  # f*[out | 1] chunks
        aepm = constp.tile([P, 32], F32)
        bepm = constp.tile([P, 32], F32)
        fpm = constp.tile([P, 32], F32)
        al1 = constp.tile([P, MB], F32)
        al2 = constp.tile([P, MB], F32)
        ysb = [constp.tile([P, MB, FOUT + 1], F32, name=f"ysb{h}")
               for h in range(2)]
        emat = constp.tile([P, 4, P], BF16)
        gcol = constp.tile([P, 2], F32)
        selg = constp.tile([P, MB, 32], F32)

        with tc.tile_pool(name="sa", bufs=2) as sa, \
             tc.tile_pool(name="xtp", bufs=4) as xtp, \
             tc.tile_pool(name="atgp", bufs=3) as atgp, \
             tc.tile_pool(name="rawp", bufs=2) as rawp, \
             tc.tile_pool(name="ps_po", bufs=2, space="PSUM") as ps_po, \
             tc.tile_pool(name="ps_y", bufs=2, space="PSUM") as ps_y, \
             tc.tile_pool(name="ps_u", bufs=2, space="PSUM") as ps_u:

            # wrhs inputs lead the sync HWDGE ring, then x chunks; all four
            # xts are dispatched up-front (bufs=4) so stage A never starves
            xts_list = []
            nc.sync.dma_start(wtile[:],
                              w_d.ap().rearrange("(c p) f -> p c f", p=P))
            nc.sync.dma_start(attb[:], attb_d.ap())
            xts0 = xtp.tile([P, 4, 16 * P], BF16, tag="xts", name="xts")
            nc.sync.dma_start(
                xts0[:, :, 0:1024],
                xt_d.ap()[:, 0:1024].rearrange("(c p) r -> p c r", p=P))
            nc.sync.dma_start(
                xts0[:, :, 1024:2048],
                xt_d.ap()[:, 1024:2048].rearrange("(c p) r -> p c r", p=P))
            xts_list.append(xts0)
            nc.sync.dma_start(emat[:], emat_d.ap())
            nc.sync.dma_start(gcol[:], g_d.ap())
            nc.sync.dma_start(selg[:], selg_d.ap())
            for s2 in range(1, 4):
                xts = xtp.tile([P, 4, 16 * P], BF16, tag="xts", name="xts")
                nc.sync.dma_start(
                    xts[:],
                    xt_d.ap()[:, s2 * 2048:(s2 + 1) * 2048].rearrange(
                        "(c p) r -> p c r", p=P))
                xts_list.append(xts)

            # build wrhs = [W | W@(a1+a2) | W@a1 | W@a2] per k-chunk, bf16
            wamf = sa.tile([P, 4, 3], F32, tag="wamf", name="wamf")
            for c in range(4):
                t1 = sa.tile([P, FOUT], F32, tag="wa_tmp", name="wa1")
                nc.vector.tensor_mul(t1[:], wtile[:, c, :], attb[:, :FOUT])
                nc.vector.tensor_reduce(wamf[:, c, 1:2], t1[:], axis=AX, op=ADD)
                t2 = sa.tile([P, FOUT], F32, tag="wa_tmp", name="wa2")
                nc.vector.tensor_mul(t2[:], wtile[:, c, :], attb[:, FOUT:])
                nc.vector.tensor_reduce(wamf[:, c, 2:3], t2[:], axis=AX, op=ADD)
                nc.vector.tensor_add(wamf[:, c, 0:1], wamf[:, c, 1:2],
                                     wamf[:, c, 2:3])
                nc.vector.tensor_copy(wrhs[:, c, :FOUT], wtile[:, c, :])
                nc.vector.tensor_copy(wrhs[:, c, FOUT:], wamf[:, c, :])

            atg_list = []

            def stage_a(g):
                xts = xts_list[g]
                usbE = sa.tile([P, 24], BF16, tag="usbE", name="usbE")
                usbO = sa.tile([P, 24], BF16, tag="usbO", name="usbO")
                for pair in range(8):
                    for half in range(2):
                        lb = 2 * pair + half      # local block 0..15
                        b = 16 * g + lb           # global block
                        po = ps_po.tile([P, FOUT + 3], F32, tag="po",
                                        name="po")
                        for c in range(4):
                            nc.tensor.matmul(
                                po[:], xts[:, c, lb * P:(lb + 1) * P],
                                wrhs[:, c, :],
                                start=(c == 0), stop=(c == 3))
                        nc.vector.tensor_copy(outb[:, b, :FOUT],
                                              po[:, :FOUT])
                        dst = usbE if half == 0 else usbO
                        nc.scalar.copy(dst[:, pair * 3:(pair + 1) * 3],
                                       po[:, FOUT:FOUT + 3])
                # parity-pick matmuls: even picks cols 0:24, odd 24:48
                pv = ps_u.tile([P, 48], F32, tag="pv", name="pv")
                nc.tensor.matmul(pv[:, 0:24], emat[:, 0, :], usbE[:],
                                 start=True, stop=False)
                nc.tensor.matmul(pv[:, 0:24], emat[:, 1, :], usbO[:],
                                 start=False, stop=True)
                nc.tensor.matmul(pv[:, 24:48], emat[:, 2, :], usbE[:],
                                 start=True, stop=False)
                nc.tensor.matmul(pv[:, 24:48], emat[:, 3, :], usbO[:],
                                 start=False, stop=True)
                pvs = sa.tile([P, 48], F32, tag="pvs", name="pvs")
                nc.vector.tensor_copy(pvs[:], pv[:])
                # abv = [ae_pre(8) | be_pre(8) | vv_pre(8)]
                abv = sa.tile([P, 24], F32, tag="abv", name="abv")
                nc.vector.tensor_copy(abv[:, 0:8], pvs[:, 0:24:3])
                nc.vector.tensor_copy(abv[:, 8:16], pvs[:, 24:48:3])
                nc.vector.tensor_add(abv[:, 16:24], pvs[:, 1:24:3],
                                     pvs[:, 26:48:3])
                tmp = sa.tile([P, 24], F32, tag="abt", name="abt")
                nc.vector.tensor_scalar_mul(tmp[:], abv[:], 0.01)
                nc.vector.tensor_max(abv[:], abv[:], tmp[:])
                abve = sa.tile([P, 24], F32, tag="abve", name="abve")
                nc.scalar.activation(abve[:], abv[:], Exp)
                sl = slice(8 * g, 8 * g + 8)
                nc.vector.tensor_copy(aepm[:, sl], abve[:, 0:8])
                nc.vector.tensor_copy(bepm[:, sl], abve[:, 8:16])
                nc.vector.tensor_scalar(fpm[:, sl], abve[:, 16:24],
                                        gcol[:, 1:2], gcol[:, 0:1],
                                        op0=MULT, op1=ADD)

            def scale_group(g):
                klo = NST * g
                m0 = klo % 32
                for t in range(NST):
                    kc = klo + t
                    m = kc % 32
                    nc.scalar.activation(outb[:, kc, :FOUT],
                                         outb[:, kc, :FOUT],
                                         mybir.ActivationFunctionType.Copy,
                                         scale=fpm[:, m:m + 1])
                nc.scalar.copy(outb[:, klo:klo + NST, FOUT:FOUT + 1],
                               fpm[:, m0:m0 + NST])

            def stage_b(g):
                klo = NST * g
                atg = atg_list[g]
                h = 0 if g < 4 else 1
                for mb in range(MB):
                    yp = ps_y.tile([P, FOUT + 1], F32, tag="yp", name="yp")
                    for t in range(NST):
                        kc = klo + t
                        nc.tensor.matmul(yp[:],
                                         atg[:, t, mb * P:(mb + 1) * P],
                                         outb[:, kc, :],
                                         start=(t == 0), stop=(t == NST - 1))
                    if g % 4 == 0:
                        nc.vector.tensor_copy(ysb[h][:, mb, :], yp[:])
                    else:
                        nc.vector.tensor_add(ysb[h][:, mb, :],
                                             ysb[h][:, mb, :], yp[:])

            # software-pipelined: stage A of super-tile s runs ahead of
            # stage B of super-tile s-1 so the PE never waits on adj DMA
            for s in range(NST):
                # adjT group: [128 j-part, 8 chunks, 1024 rows]. Chunks 0-4
                # via SWDGE cast-DMA (i32 -> bf16, ~270 GB/s cap); chunks 5-7
                # raw i32 on the scalar HWDGE ring (separate FIFO from sync)
                # + DVE convert — the DMA paths run concurrently so the adj
                # stream is not conversion-bound.
                atg = atgp.tile([P, NST, RPC], BF16, tag="atg", name="atg")
                for hh in range(2):
                    rs = slice(hh * 512, (hh + 1) * 512)
                    nc.gpsimd.dma_start(
                        atg[:, 0:5, rs],
                        adjt_d.ap()[s * 1024:s * 1024 + 5 * P, rs].rearrange(
                            "(t p) r -> p t r", p=P))
                raw = rawp.tile([P, 3, RPC], I32, tag="raw", name="raw")
                nc.scalar.dma_start(
                    raw[:],
                    adjt_d.ap()[s * 1024 + 5 * P:(s + 1) * 1024, :].rearrange(
                        "(t p) r -> p t r", p=P))
                nc.vector.tensor_copy(atg[:, 5:8, :], raw[:])
                atg_list.append(atg)

                if s < 4:
                    stage_a(s)
                scale_group(s)
                if s >= 1:
                    stage_b(s - 1)
            stage_b(NST - 1)

            # ---- alphas ----
            for mb in range(MB):
                m1 = sa.tile([P, 32], F32, tag="alm", name="alm1")
                nc.vector.tensor_mul(m1[:], aepm[:], selg[:, mb, :])
                nc.vector.tensor_reduce(al1[:, mb:mb + 1], m1[:], axis=AX,
                                        op=ADD)
                m2 = sa.tile([P, 32], F32, tag="alm", name="alm2")
                nc.vector.tensor_mul(m2[:], bepm[:], selg[:, mb, :])
                nc.vector.tensor_reduce(al2[:, mb:mb + 1], m2[:], axis=AX,
                                        op=ADD)
            nc.vector.tensor_scalar_add(al1[:], al1[:], gcol[:, 1:2])
            nc.vector.tensor_scalar_add(al2[:], al2[:], gcol[:, 1:2])

            # ---- combine + sigmoid + store ----
            for mb in range(MB):
                z1 = sa.tile([P, FOUT + 1], F32, tag="z1", name="z1")
                z2 = sa.tile([P, FOUT + 1], F32, tag="z2", name="z2")
                nc.vector.tensor_scalar_mul(z1[:], ysb[0][:, mb, :],
                                            al1[:, mb:mb + 1])
                nc.vector.tensor_scalar_mul(z2[:], ysb[1][:, mb, :],
                                            al2[:, mb:mb + 1])
                nc.vector.tensor_add(z1[:], z1[:], z2[:])
                rec = sa.tile([P, 1], F32, tag="rec", name="rec")
                nc.vector.reciprocal(rec[:], z1[:, FOUT:FOUT + 1])
                res = sa.tile([P, FOUT], F32, tag="res", name="res")
                nc.vector.tensor_scalar_mul(res[:], z1[:, :FOUT], rec[:])
                resg = sa.tile([P, FOUT], F32, tag="resg", name="resg")
                nc.scalar.activation(resg[:], res[:], Sigmoid)
                nc.sync.dma_start(y_d.ap()[mb * P:(mb + 1) * P, :], resg[:])

    nc.compile()
    return nc


_NC_CACHE = None


def _get_program():
    global _NC_CACHE
    if _NC_CACHE is None:
        _NC_CACHE = build_program()
    return _NC_CACHE


def _to_bf16(a):
    return np.ascontiguousarray(np.asarray(a, np.float32)).astype(
        ml_dtypes.bfloat16)


def make_in_maps(x, weight, att_vec, adj):
    x = np.asarray(x, dtype=np.float32)
    weight = np.ascontiguousarray(np.asarray(weight, dtype=np.float32))
    att_vec = np.asarray(att_vec, dtype=np.float32)
    adj = np.asarray(adj, dtype=np.int32)

    xt = _to_bf16(x.T)                             # [512, 8192] bf16
    attb = np.broadcast_to(att_vec[:, 0][None, :], (P, 2 * FOUT)).copy()
    adjT = np.ascontiguousarray(adj.T)             # [8192, 8192] int32

    emat = np.zeros((P, 4, P), np.float32)
    for i in range(P):
        if i < 64:
            emat[2 * i, 0, i] = 1
            emat[2 * i + 1, 2, i] = 1
        else:
            emat[2 * i - 128, 1, i] = 1
            emat[2 * i - 127, 3, i] = 1
    emat = _to_bf16(emat)

    in_maps = []
    for c in range(NCORES):
        g = 1.0 if c < 4 else 0.0
        gcol = np.empty((P, 2), np.float32)
        gcol[:, 0] = g
        gcol[:, 1] = 1.0 - g
        selg = np.zeros((P, MB, 32), np.float32)
        for mb in range(MB):
            selg[:, mb, (8 * c + mb) % 32] = g
        in_maps.append({
            "xt": xt,
            "w": weight,
            "attb": attb,
            "adjt": np.ascontiguousarray(adjT[:, c * RPC:(c + 1) * RPC]),
            "gcol": gcol,
            "selg": selg,
            "emat": emat,
        })
    return in_maps


def kernel(x, weight, att_vec, adj, _trace=False, _trace_kwargs=None):
    nc = _get_program()
    in_maps = make_in_maps(x, weight, att_vec, adj)
    r = run_bass_kernel_spmd(nc, in_maps, core_ids=list(range(NCORES)),
                             trace=_trace, **(_trace_kwargs or {}))
    y = np.concatenate([r.results[c]["y"] for c in range(NCORES)], axis=0)
    kernel.last_results = r
    return y.astype(np.float32)


# revision 25
# speedup vs baseline: 1.0506x; 1.0506x over previous
"""GAT layer (nn_GAT_layer_67619965108552) as a Trainium2 Bass/Tile SPMD kernel.

Structure exploited (validated vs reference to 5.2e-3 in bf16):
  With n=8192, the buggy-but-faithful pair indexing collapses:
    rows i < 4096:  scores[i, j] = u[2i + (j >= 4096)],  u = x @ (W@a1 + W@a2)
    rows i >= 4096: scores[i, j] = tt[j mod 4096],  tt[q] = s1[2q] + s2[2q+1]
  After leaky_relu + adj masking + softmax, attn @ out reduces to masked
  row-sum matmuls against f-scaled out:
    ysb[h][mb] = sum_{kc in half h} adjT(mb,kc)^T @ (f * [out | 1])[kc]
    res = sigmoid((al1*ysbL + al2*ysbR)[:, :256] / (...)[:, 256])
  Top-half cores: f = 1, al1/al2 = exp(lrelu(u at even/odd rows)); bottom-half
  cores: f = exp(lrelu(tt)), al = 1. One instruction stream for all cores;
  divergence is data-driven (g flag / select masks).

Single fused pipeline in 8 super-tiles: x^T (bf16) streams on the HWDGE queue
while the per-core transposed adj slice (int32 -> bf16 cast) streams on the
SWDGE queue in 4MB column-group DMAs with 4KB-contiguous runs — no on-chip
transposes at all; u/s1/s2 come free as 3 extra matmul columns; per-pair
score vectors are extracted with constant 0/1 parity-pick matmuls; f-scaling
is per-chunk so nothing serializes globally.

Sharding: rows of adj / output across 8 cores (1024 each); x/weight/att_vec
replicated (each core computes the full out = x@W as the rhs of its matmuls).
"""
import ml_dtypes
import numpy as np
from contextlib import ExitStack

import concourse.bass as bass
import concourse.tile as tile
from concourse import bacc, mybir
from concourse.bass_utils import run_bass_kernel_spmd

F32 = mybir.dt.float32
BF16 = mybir.dt.bfloat16
I32 = mybir.dt.int32

N = 8192           # nodes
FIN = 512          # input features
FOUT = 256         # output features
P = 128
NB = N // P        # 64 row-blocks of out
NCORES = 8
RPC = N // NCORES  # 1024 rows per core
MB = RPC // P      # 8 output row-blocks per core
NST = 8            # super-tiles (adj column groups of 1024)


def build_program():
    nc = bacc.Bacc("TRN2", target_bir_lowering=False, debug=False,
                   num_devices=NCORES)

    xt_d = nc.dram_tensor("xt", [FIN, N], BF16, kind="ExternalInput")
    w_d = nc.dram_tensor("w", [FIN, FOUT], F32, kind="ExternalInput")
    attb_d = nc.dram_tensor("attb", [P, 2 * FOUT], F32, kind="ExternalInput")
    # per-core transposed adj slice, partition-contiguous layout:
    # adjt[s*128 + p, t*1024 + r] = adj[c*RPC + r, s*1024 + t*128 + p]
    adjt_d = nc.dram_tensor("adjt", [NST * P, NST * RPC], I32,
                            kind="ExternalInput")
    # gcol[:, 0] = g (1.0 top-half cores, 0.0 bottom), gcol[:, 1] = 1-g
    g_d = nc.dram_tensor("gcol", [P, 2], F32, kind="ExternalInput")
    # selg[p, mb, m] = g * (m == 8c + mb) : per-core pair select for alphas
    selg_d = nc.dram_tensor("selg", [P, MB, 32], F32, kind="ExternalInput")
    # parity-pick matrices [E0 | E1 | Eo0 | Eo1]
    emat_d = nc.dram_tensor("emat", [P, 4, P], BF16, kind="ExternalInput")
    y_d = nc.dram_tensor("y", [RPC, FOUT], F32, kind="ExternalOutput")

    Exp = mybir.ActivationFunctionType.Exp
    Sigmoid = mybir.ActivationFunctionType.Sigmoid
    AX = mybir.AxisListType.X
    ADD = mybir.AluOpType.add
    MULT = mybir.AluOpType.mult

    with tile.TileContext(nc) as tc, ExitStack() as ctx:
        constp = ctx.enter_context(tc.tile_pool(name="const", bufs=1))

        wtile = constp.tile([P, 4, FOUT], F32)
        attb = constp.tile([P, 2 * FOUT], F32)

        # persistent tensors
        wrhs = constp.tile([P, 4, FOUT + 3], BF16)   # [W | wu | w1 | w2]
        outb = constp.tile([P, NB, FOUT + 1], BF16)  # f*[out | 1] chunks
        aepm = constp.tile([P, 32], F32)
        bepm = constp.tile([P, 32], F32)
        fpm = constp.tile([P, 32], F32)
        al1 = constp.tile([P, MB], F32)
        al2 = constp.tile([P, MB], F32)
        ysb = [constp.tile([P, MB, FOUT + 1], F32, name=f"ysb{h}")
               for h in range(2)]
        emat = constp.tile([P, 4, P], BF16)
        gcol = constp.tile([P, 2], F32)
        selg = constp.tile([P, MB, 32], F32)

        with tc.tile_pool(name="sa", bufs=3) as sa, \
             tc.tile_pool(name="xtp", bufs=4) as xtp, \
             tc.tile_pool(name="atgp", bufs=4) as atgp, \
             tc.tile_pool(name="ps_po", bufs=2, space="PSUM") as ps_po, \
             tc.tile_pool(name="ps_y", bufs=2, space="PSUM") as ps_y, \
             tc.tile_pool(name="ps_u", bufs=2, space="PSUM") as ps_u:

            # wrhs inputs lead the sync HWDGE ring, then x chunks; all four
            # xts are dispatched up-front (bufs=4) so stage A never starves
            xts_list = []
            nc.sync.dma_start(wtile[:],
                              w_d.ap().rearrange("(c p) f -> p c f", p=P))
            nc.sync.dma_start(attb[:], attb_d.ap())
            xts0 = xtp.tile([P, 4, 16 * P], BF16, tag="xts", name="xts")
            nc.sync.dma_start(
                xts0[:, :, 0:1024],
                xt_d.ap()[:, 0:1024].rearrange("(c p) r -> p c r", p=P))
            nc.sync.dma_start(
                xts0[:, :, 1024:2048],
                xt_d.ap()[:, 1024:2048].rearrange("(c p) r -> p c r", p=P))
            xts_list.append(xts0)
            nc.sync.dma_start(emat[:], emat_d.ap())
            nc.sync.dma_start(gcol[:], g_d.ap())
            nc.sync.dma_start(selg[:], selg_d.ap())
            for s2 in range(1, 4):
                xts = xtp.tile([P, 4, 16 * P], BF16, tag="xts", name="xts")
                nc.sync.dma_start(
                    xts[:],
                    xt_d.ap()[:, s2 * 2048:(s2 + 1) * 2048].rearrange(
                        "(c p) r -> p c r", p=P))
                xts_list.append(xts)

            # build wrhs = [W | W@(a1+a2) | W@a1 | W@a2] per k-chunk, bf16
            wamf = sa.tile([P, 4, 3], F32, tag="wamf", name="wamf")
            for c in range(4):
                t1 = sa.tile([P, FOUT], F32, tag="wa_tmp", name="wa1")
                nc.vector.tensor_mul(t1[:], wtile[:, c, :], attb[:, :FOUT])
                nc.vector.tensor_reduce(wamf[:, c, 1:2], t1[:], axis=AX, op=ADD)
                t2 = sa.tile([P, FOUT], F32, tag="wa_tmp", name="wa2")
                nc.vector.tensor_mul(t2[:], wtile[:, c, :], attb[:, FOUT:])
                nc.vector.tensor_reduce(wamf[:, c, 2:3], t2[:], axis=AX, op=ADD)
                nc.vector.tensor_add(wamf[:, c, 0:1], wamf[:, c, 1:2],
                                     wamf[:, c, 2:3])
                nc.vector.tensor_copy(wrhs[:, c, :FOUT], wtile[:, c, :])
                nc.vector.tensor_copy(wrhs[:, c, FOUT:], wamf[:, c, :])

            atg_list = []

            def stage_a(g):
                xts = xts_list[g]
                usbE = sa.tile([P, 24], BF16, tag="usbE", name="usbE")
                usbO = sa.tile([P, 24], BF16, tag="usbO", name="usbO")
                for pair in range(8):
                    for half in range(2):
                        lb = 2 * pair + half      # local block 0..15
                        b = 16 * g + lb           # global block
                        po = ps_po.tile([P, FOUT + 3], F32, tag="po",
                                        name="po")
                        for c in range(4):
                            nc.tensor.matmul(
                                po[:], xts[:, c, lb * P:(lb + 1) * P],
                                wrhs[:, c, :],
                                start=(c == 0), stop=(c == 3))
                        nc.vector.tensor_copy(outb[:, b, :FOUT],
                                              po[:, :FOUT])
                        dst = usbE if half == 0 else usbO
                        nc.scalar.copy(dst[:, pair * 3:(pair + 1) * 3],
                                       po[:, FOUT:FOUT + 3])
                # parity-pick matmuls: even picks cols 0:24, odd 24:48
                pv = ps_u.tile([P, 48], F32, tag="pv", name="pv")
                nc.tensor.matmul(pv[:, 0:24], emat[:, 0, :], usbE[:],
                                 start=True, stop=False)
                nc.tensor.matmul(pv[:, 0:24], emat[:, 1, :], usbO[:],
                                 start=False, stop=True)
                nc.tensor.matmul(pv[:, 24:48], emat[:, 2, :], usbE[:],
                                 start=True, stop=False)
                nc.tensor.matmul(pv[:, 24:48], emat[:, 3, :], usbO[:],
                                 start=False, stop=True)
                pvs = sa.tile([P, 48], F32, tag="pvs", name="pvs")
                nc.vector.tensor_copy(pvs[:], pv[:])
                # abv = [ae_pre(8) | be_pre(8) | vv_pre(8)]
                abv = sa.tile([P, 24], F32, tag="abv", name="abv")
                nc.vector.tensor_copy(abv[:, 0:8], pvs[:, 0:24:3])
                nc.vector.tensor_copy(abv[:, 8:16], pvs[:, 24:48:3])
                nc.vector.tensor_add(abv[:, 16:24], pvs[:, 1:24:3],
                                     pvs[:, 26:48:3])
                tmp = sa.tile([P, 24], F32, tag="abt", name="abt")
                nc.vector.tensor_scalar_mul(tmp[:], abv[:], 0.01)
                nc.vector.tensor_max(abv[:], abv[:], tmp[:])
                abve = sa.tile([P, 24], F32, tag="abve", name="abve")
                nc.scalar.activation(abve[:], abv[:], Exp)
                sl = slice(8 * g, 8 * g + 8)
                nc.vector.tensor_copy(aepm[:, sl], abve[:, 0:8])
                nc.vector.tensor_copy(bepm[:, sl], abve[:, 8:16])
                nc.vector.tensor_scalar(fpm[:, sl], abve[:, 16:24],
                                        gcol[:, 1:2], gcol[:, 0:1],
                                        op0=MULT, op1=ADD)

            def scale_group(g):
                klo = NST * g
                m0 = klo % 32
                for t in range(NST):
                    kc = klo + t
                    m = kc % 32
                    nc.scalar.activation(outb[:, kc, :FOUT],
                                         outb[:, kc, :FOUT],
                                         mybir.ActivationFunctionType.Copy,
                                         scale=fpm[:, m:m + 1])
                nc.scalar.copy(outb[:, klo:klo + NST, FOUT:FOUT + 1],
                               fpm[:, m0:m0 + NST])

            def combine_store(mb, y2):
                # z = al1*ysbL + al2*ysbR; res = sigmoid(z[:,:256]/z[:,256])
                z1 = sa.tile([P, FOUT + 1], F32, tag="z1", name="z1")
                nc.vector.tensor_scalar_mul(z1[:], ysb[0][:, mb, :],
                                            al1[:, mb:mb + 1])
                z2 = sa.tile([P, FOUT + 1], F32, tag="z2", name="z2")
                nc.vector.tensor_scalar_mul(z2[:], y2, al2[:, mb:mb + 1])
                nc.vector.tensor_add(z1[:], z1[:], z2[:])
                rec = sa.tile([P, 1], F32, tag="rec", name="rec")
                nc.vector.reciprocal(rec[:], z1[:, FOUT:FOUT + 1])
                res = sa.tile([P, FOUT], F32, tag="res", name="res")
                nc.vector.tensor_scalar_mul(res[:], z1[:, :FOUT], rec[:])
                resg = sa.tile([P, FOUT], F32, tag="resg", name="resg")
                nc.scalar.activation(resg[:], res[:], Sigmoid)
                nc.sync.dma_start(y_d.ap()[mb * P:(mb + 1) * P, :], resg[:])

            def stage_b(g):
                klo = NST * g
                atg = atg_list[g]
                h = 0 if g < 4 else 1
                for mb in range(MB):
                    yp = ps_y.tile([P, FOUT + 1], F32, tag="yp", name="yp")
                    for t in range(NST):
                        kc = klo + t
                        nc.tensor.matmul(yp[:],
                                         atg[:, t, mb * P:(mb + 1) * P],
                                         outb[:, kc, :],
                                         start=(t == 0), stop=(t == NST - 1))
                    if g % 4 == 0:
                        nc.vector.tensor_copy(ysb[h][:, mb, :], yp[:])
                    elif g < NST - 1:
                        nc.vector.tensor_add(ysb[h][:, mb, :],
                                             ysb[h][:, mb, :], yp[:])
                    else:
                        # final group: fold combine+sigmoid+store per mb
                        y2 = sa.tile([P, FOUT + 1], F32, tag="y2", name="y2")
                        nc.vector.tensor_add(y2[:], ysb[1][:, mb, :], yp[:])
                        combine_store(mb, y2[:])

            def compute_alphas():
                for mb in range(MB):
                    m1 = sa.tile([P, 32], F32, tag="alm", name="alm1")
                    nc.vector.tensor_mul(m1[:], aepm[:], selg[:, mb, :])
                    nc.vector.tensor_reduce(al1[:, mb:mb + 1], m1[:], axis=AX,
                                            op=ADD)
                    m2 = sa.tile([P, 32], F32, tag="alm", name="alm2")
                    nc.vector.tensor_mul(m2[:], bepm[:], selg[:, mb, :])
                    nc.vector.tensor_reduce(al2[:, mb:mb + 1], m2[:], axis=AX,
                                            op=ADD)
                nc.vector.tensor_scalar_add(al1[:], al1[:], gcol[:, 1:2])
                nc.vector.tensor_scalar_add(al2[:], al2[:], gcol[:, 1:2])

            # software-pipelined: stage A of super-tile s runs ahead of
            # stage B of super-tile s-1 so the PE never waits on adj DMA
            for s in range(NST):
                # adjT group: [128 j-part, 8 chunks, 1024 rows], i32 -> bf16
                # SWDGE cast-DMA; one contiguous run per partition per piece
                atg = atgp.tile([P, NST, RPC], BF16, tag="atg", name="atg")
                for hh in range(2):
                    ts = slice(hh * 4, (hh + 1) * 4)
                    nc.gpsimd.dma_start(
                        atg[:, ts, :],
                        adjt_d.ap()[s * P:(s + 1) * P,
                                    hh * 4096:(hh + 1) * 4096].rearrange(
                            "p (t r) -> p t r", t=4))
                atg_list.append(atg)

                if s < 4:
                    stage_a(s)
                scale_group(s)
                if s == 3:
                    compute_alphas()
                if s >= 1:
                    stage_b(s - 1)
            stage_b(NST - 1)

    nc.compile()
    return nc


_NC_CACHE = None


def _get_program():
    global _NC_CACHE
    if _NC_CACHE is None:
        _NC_CACHE = build_program()
    return _NC_CACHE


def _to_bf16(a):
    return np.ascontiguousarray(np.asarray(a, np.float32)).astype(
        ml_dtypes.bfloat16)


def make_in_maps(x, weight, att_vec, adj):
    x = np.asarray(x, dtype=np.float32)
    weight = np.ascontiguousarray(np.asarray(weight, dtype=np.float32))
    att_vec = np.asarray(att_vec, dtype=np.float32)
    adj = np.asarray(adj, dtype=np.int32)

    xt = _to_bf16(x.T)                             # [512, 8192] bf16
    attb = np.broadcast_to(att_vec[:, 0][None, :], (P, 2 * FOUT)).copy()
    adjT = adj.T                                   # [8192, 8192] int32 view

    emat = np.zeros((P, 4, P), np.float32)
    for i in range(P):
        if i < 64:
            emat[2 * i, 0, i] = 1
            emat[2 * i + 1, 2, i] = 1
        else:
            emat[2 * i - 128, 1, i] = 1
            emat[2 * i - 127, 3, i] = 1
    emat = _to_bf16(emat)

    in_maps = []
    for c in range(NCORES):
        g = 1.0 if c < 4 else 0.0
        gcol = np.empty((P, 2), np.float32)
        gcol[:, 0] = g
        gcol[:, 1] = 1.0 - g
        selg = np.zeros((P, MB, 32), np.float32)
        for mb in range(MB):
            selg[:, mb, (8 * c + mb) % 32] = g
        in_maps.append({
            "xt": xt,
            "w": weight,
            "attb": attb,
            # (s, t, p, r) -> (s, p, t, r): one contiguous 32KB run per
            # SBUF partition per super-tile
            "adjt": np.ascontiguousarray(
                adjT[:, c * RPC:(c + 1) * RPC]
                .reshape(NST, NST, P, RPC)
                .transpose(0, 2, 1, 3)
                .reshape(NST * P, NST * RPC)),
            "gcol": gcol,
            "selg": selg,
            "emat": emat,
        })
    return in_maps


def kernel(x, weight, att_vec, adj, _trace=False, _trace_kwargs=None):
    nc = _get_program()
    in_maps = make_in_maps(x, weight, att_vec, adj)
    r = run_bass_kernel_spmd(nc, in_maps, core_ids=list(range(NCORES)),
                             trace=_trace, **(_trace_kwargs or {}))
    y = np.concatenate([r.results[c]["y"] for c in range(NCORES)], axis=0)
    kernel.last_results = r
    return y.astype(np.float32)


# revision 28
# speedup vs baseline: 1.1939x; 1.1364x over previous
"""GAT layer (nn_GAT_layer_67619965108552) as a Trainium2 Bass/Tile SPMD kernel.

Structure exploited (validated vs reference to 5.2e-3 in bf16):
  With n=8192, the buggy-but-faithful pair indexing collapses:
    rows i < 4096:  scores[i, j] = u[2i + (j >= 4096)],  u = x @ (W@a1 + W@a2)
    rows i >= 4096: scores[i, j] = tt[j mod 4096],  tt[q] = s1[2q] + s2[2q+1]
  After leaky_relu + adj masking + softmax, attn @ out reduces to masked
  row-sum matmuls against f-scaled out:
    ysb[h][mb] = sum_{kc in half h} adjT(mb,kc)^T @ (f * [out | 1])[kc]
    res = sigmoid((al1*ysbL + al2*ysbR)[:, :256] / (...)[:, 256])
  Top-half cores: f = 1, al1/al2 = exp(lrelu(u at even/odd rows)); bottom-half
  cores: f = exp(lrelu(tt)), al = 1. One instruction stream for all cores;
  divergence is data-driven (g flag / select masks).

Single fused pipeline in 8 super-tiles: x^T (bf16) streams on the HWDGE queue
while the per-core transposed adj slice (int32 -> bf16 cast) streams on the
SWDGE queue in 4MB column-group DMAs with 4KB-contiguous runs — no on-chip
transposes at all; u/s1/s2 come free as 3 extra matmul columns; per-pair
score vectors are extracted with constant 0/1 parity-pick matmuls; f-scaling
is per-chunk so nothing serializes globally.

Sharding: rows of adj / output across 8 cores (1024 each); x/weight/att_vec
replicated (each core computes the full out = x@W as the rhs of its matmuls).
"""
import ml_dtypes
import numpy as np
from contextlib import ExitStack

import concourse.bass as bass
import concourse.tile as tile
from concourse import bacc, mybir
from concourse.bass_utils import run_bass_kernel_spmd

F32 = mybir.dt.float32
BF16 = mybir.dt.bfloat16
I32 = mybir.dt.int32

N = 8192           # nodes
FIN = 512          # input features
FOUT = 256         # output features
P = 128
NB = N // P        # 64 row-blocks of out
NCORES = 8
RPC = N // NCORES  # 1024 rows per core
MB = RPC // P      # 8 output row-blocks per core
NST = 8            # super-tiles (adj column groups of 1024)


def build_program():
    nc = bacc.Bacc("TRN2", target_bir_lowering=False, debug=False,
                   num_devices=NCORES)

    xt_d = nc.dram_tensor("xt", [FIN, N], BF16, kind="ExternalInput")
    w_d = nc.dram_tensor("w", [FIN, FOUT], F32, kind="ExternalInput")
    attb_d = nc.dram_tensor("attb", [P, 2 * FOUT], F32, kind="ExternalInput")
    # per-core transposed adj slice, partition-contiguous layout:
    # adjt[s*128 + p, t*1024 + r] = adj[c*RPC + r, s*1024 + t*128 + p]
    adjt_d = nc.dram_tensor("adjt", [NST * P, NST * RPC], I32,
                            kind="ExternalInput")
    # gcol[:, 0] = g (1.0 top-half cores, 0.0 bottom), gcol[:, 1] = 1-g
    g_d = nc.dram_tensor("gcol", [P, 2], F32, kind="ExternalInput")
    # selg[p, mb, m] = g * (m == 8c + mb) : per-core pair select for alphas
    selg_d = nc.dram_tensor("selg", [P, MB, 32], F32, kind="ExternalInput")
    # parity-pick matrices [E0 | E1 | Eo0 | Eo1]
    emat_d = nc.dram_tensor("emat", [P, 4, P], BF16, kind="ExternalInput")
    y_d = nc.dram_tensor("y", [RPC, FOUT], F32, kind="ExternalOutput")

    Exp = mybir.ActivationFunctionType.Exp
    Sigmoid = mybir.ActivationFunctionType.Sigmoid
    AX = mybir.AxisListType.X
    ADD = mybir.AluOpType.add
    MULT = mybir.AluOpType.mult

    with tile.TileContext(nc) as tc, ExitStack() as ctx:
        constp = ctx.enter_context(tc.tile_pool(name="const", bufs=1))

        wtile = constp.tile([P, 4, FOUT], F32)
        attb = constp.tile([P, 2 * FOUT], F32)

        # persistent tensors
        wrhs = constp.tile([P, 4, FOUT + 3], BF16)   # [W | wu | w1 | w2]
        outb = constp.tile([P, NB, FOUT + 1], BF16)  # f*[out | 1] chunks
        aepm = constp.tile([P, 32], F32)
        bepm = constp.tile([P, 32], F32)
        fpm = constp.tile([P, 32], F32)
        al1 = constp.tile([P, MB], F32)
        al2 = constp.tile([P, MB], F32)
        ysb = [constp.tile([P, MB, FOUT + 1], F32, name=f"ysb{h}")
               for h in range(2)]
        emat = constp.tile([P, 4, P], BF16)
        gcol = constp.tile([P, 2], F32)
        selg = constp.tile([P, MB, 32], F32)

        with tc.tile_pool(name="sa", bufs=3) as sa, \
             tc.tile_pool(name="xtp", bufs=4) as xtp, \
             tc.tile_pool(name="atgp", bufs=4) as atgp, \
             tc.tile_pool(name="ps_po", bufs=2, space="PSUM") as ps_po, \
             tc.tile_pool(name="ps_y", bufs=2, space="PSUM") as ps_y, \
             tc.tile_pool(name="ps_u", bufs=2, space="PSUM") as ps_u:

            # Single DMA queue (SWDGE) carries everything in explicit
            # priority order — the two HWDGE/SWDGE queues share the same 16
            # SDMA engines, so two active queues just fight. Tiny consts ride
            # the otherwise-idle sync ring.
            nc.sync.dma_start(emat[:], emat_d.ap())
            nc.sync.dma_start(gcol[:], g_d.ap())
            nc.sync.dma_start(selg[:], selg_d.ap())

            xts_list = []
            atg_list = []

            def emit_xts(s2, half=None):
                if half is None or half == 0:
                    xts = xtp.tile([P, 4, 16 * P], BF16, tag="xts",
                                   name="xts")
                    xts_list.append(xts)
                xts = xts_list[s2]
                lo = 0 if half in (None, 0) else 1024
                hi = 2048 if half in (None, 1) else 1024
                nc.gpsimd.dma_start(
                    xts[:, :, lo:hi],
                    xt_d.ap()[:, s2 * 2048 + lo:s2 * 2048 + hi].rearrange(
                        "(c p) r -> p c r", p=P))

            def emit_atg(s2, hh):
                if hh == 0:
                    atg_list.append(atgp.tile([P, NST, RPC], BF16, tag="atg",
                                              name="atg"))
                atg = atg_list[s2]
                nc.gpsimd.dma_start(
                    atg[:, hh * 4:(hh + 1) * 4, :],
                    adjt_d.ap()[s2 * P:(s2 + 1) * P,
                                hh * 4096:(hh + 1) * 4096].rearrange(
                        "p (t r) -> p t r", t=4))

            nc.gpsimd.dma_start(wtile[:],
                                w_d.ap().rearrange("(c p) f -> p c f", p=P))
            nc.gpsimd.dma_start(attb[:], attb_d.ap())
            emit_xts(0, 0)
            emit_xts(0, 1)
            emit_atg(0, 0)
            emit_xts(1)
            emit_atg(0, 1)
            emit_xts(2)
            emit_atg(1, 0)
            emit_atg(1, 1)
            emit_xts(3)
            for s2 in range(2, NST):
                emit_atg(s2, 0)
                emit_atg(s2, 1)

            # build wrhs = [W | W@(a1+a2) | W@a1 | W@a2] per k-chunk, bf16
            wamf = sa.tile([P, 4, 3], F32, tag="wamf", name="wamf")
            for c in range(4):
                t1 = sa.tile([P, FOUT], F32, tag="wa_tmp", name="wa1")
                nc.vector.tensor_mul(t1[:], wtile[:, c, :], attb[:, :FOUT])
                nc.vector.tensor_reduce(wamf[:, c, 1:2], t1[:], axis=AX, op=ADD)
                t2 = sa.tile([P, FOUT], F32, tag="wa_tmp", name="wa2")
                nc.vector.tensor_mul(t2[:], wtile[:, c, :], attb[:, FOUT:])
                nc.vector.tensor_reduce(wamf[:, c, 2:3], t2[:], axis=AX, op=ADD)
                nc.vector.tensor_add(wamf[:, c, 0:1], wamf[:, c, 1:2],
                                     wamf[:, c, 2:3])
                nc.vector.tensor_copy(wrhs[:, c, :FOUT], wtile[:, c, :])
                nc.vector.tensor_copy(wrhs[:, c, FOUT:], wamf[:, c, :])

            def stage_a(g):
                xts = xts_list[g]
                usbE = sa.tile([P, 24], BF16, tag="usbE", name="usbE")
                usbO = sa.tile([P, 24], BF16, tag="usbO", name="usbO")
                for pair in range(8):
                    for half in range(2):
                        lb = 2 * pair + half      # local block 0..15
                        b = 16 * g + lb           # global block
                        po = ps_po.tile([P, FOUT + 3], F32, tag="po",
                                        name="po")
                        for c in range(4):
                            nc.tensor.matmul(
                                po[:], xts[:, c, lb * P:(lb + 1) * P],
                                wrhs[:, c, :],
                                start=(c == 0), stop=(c == 3))
                        nc.vector.tensor_copy(outb[:, b, :FOUT],
                                              po[:, :FOUT])
                        dst = usbE if half == 0 else usbO
                        nc.scalar.copy(dst[:, pair * 3:(pair + 1) * 3],
                                       po[:, FOUT:FOUT + 3])
                # parity-pick matmuls: even picks cols 0:24, odd 24:48
                pv = ps_u.tile([P, 48], F32, tag="pv", name="pv")
                nc.tensor.matmul(pv[:, 0:24], emat[:, 0, :], usbE[:],
                                 start=True, stop=False)
                nc.tensor.matmul(pv[:, 0:24], emat[:, 1, :], usbO[:],
                                 start=False, stop=True)
                nc.tensor.matmul(pv[:, 24:48], emat[:, 2, :], usbE[:],
                                 start=True, stop=False)
                nc.tensor.matmul(pv[:, 24:48], emat[:, 3, :], usbO[:],
                                 start=False, stop=True)
                pvs = sa.tile([P, 48], F32, tag="pvs", name="pvs")
                nc.vector.tensor_copy(pvs[:], pv[:])
                # abv = [ae_pre(8) | be_pre(8) | vv_pre(8)]
                abv = sa.tile([P, 24], F32, tag="abv", name="abv")
                nc.vector.tensor_copy(abv[:, 0:8], pvs[:, 0:24:3])
                nc.vector.tensor_copy(abv[:, 8:16], pvs[:, 24:48:3])
                nc.vector.tensor_add(abv[:, 16:24], pvs[:, 1:24:3],
                                     pvs[:, 26:48:3])
                tmp = sa.tile([P, 24], F32, tag="abt", name="abt")
                nc.vector.tensor_scalar_mul(tmp[:], abv[:], 0.01)
                nc.vector.tensor_max(abv[:], abv[:], tmp[:])
                abve = sa.tile([P, 24], F32, tag="abve", name="abve")
                nc.scalar.activation(abve[:], abv[:], Exp)
                sl = slice(8 * g, 8 * g + 8)
                nc.vector.tensor_copy(aepm[:, sl], abve[:, 0:8])
                nc.vector.tensor_copy(bepm[:, sl], abve[:, 8:16])
                nc.vector.tensor_scalar(fpm[:, sl], abve[:, 16:24],
                                        gcol[:, 1:2], gcol[:, 0:1],
                                        op0=MULT, op1=ADD)

            def scale_group(g):
                klo = NST * g
                m0 = klo % 32
                for t in range(NST):
                    kc = klo + t
                    m = kc % 32
                    nc.scalar.activation(outb[:, kc, :FOUT],
                                         outb[:, kc, :FOUT],
                                         mybir.ActivationFunctionType.Copy,
                                         scale=fpm[:, m:m + 1])
                nc.scalar.copy(outb[:, klo:klo + NST, FOUT:FOUT + 1],
                               fpm[:, m0:m0 + NST])

            def combine_store(mb, y2):
                # z = al1*ysbL + al2*ysbR; res = sigmoid(z[:,:256]/z[:,256])
                z1 = sa.tile([P, FOUT + 1], F32, tag="z1", name="z1")
                nc.vector.tensor_scalar_mul(z1[:], ysb[0][:, mb, :],
                                            al1[:, mb:mb + 1])
                z2 = sa.tile([P, FOUT + 1], F32, tag="z2", name="z2")
                nc.vector.tensor_scalar_mul(z2[:], y2, al2[:, mb:mb + 1])
                nc.vector.tensor_add(z1[:], z1[:], z2[:])
                rec = sa.tile([P, 1], F32, tag="rec", name="rec")
                nc.vector.reciprocal(rec[:], z1[:, FOUT:FOUT + 1])
                res = sa.tile([P, FOUT], F32, tag="res", name="res")
                nc.vector.tensor_scalar_mul(res[:], z1[:, :FOUT], rec[:])
                resg = sa.tile([P, FOUT], F32, tag="resg", name="resg")
                nc.scalar.activation(resg[:], res[:], Sigmoid)
                nc.sync.dma_start(y_d.ap()[mb * P:(mb + 1) * P, :], resg[:])

            def stage_b(g):
                klo = NST * g
                atg = atg_list[g]
                h = 0 if g < 4 else 1
                for mb in range(MB):
                    yp = ps_y.tile([P, FOUT + 1], F32, tag="yp", name="yp")
                    for t in range(NST):
                        kc = klo + t
                        nc.tensor.matmul(yp[:],
                                         atg[:, t, mb * P:(mb + 1) * P],
                                         outb[:, kc, :],
                                         start=(t == 0), stop=(t == NST - 1))
                    if g % 4 == 0:
                        nc.vector.tensor_copy(ysb[h][:, mb, :], yp[:])
                    elif g < NST - 1:
                        nc.vector.tensor_add(ysb[h][:, mb, :],
                                             ysb[h][:, mb, :], yp[:])
                    else:
                        # final group: fold combine+sigmoid+store per mb
                        y2 = sa.tile([P, FOUT + 1], F32, tag="y2", name="y2")
                        nc.vector.tensor_add(y2[:], ysb[1][:, mb, :], yp[:])
                        combine_store(mb, y2[:])

            def compute_alphas():
                for mb in range(MB):
                    m1 = sa.tile([P, 32], F32, tag="alm", name="alm1")
                    nc.vector.tensor_mul(m1[:], aepm[:], selg[:, mb, :])
                    nc.vector.tensor_reduce(al1[:, mb:mb + 1], m1[:], axis=AX,
                                            op=ADD)
                    m2 = sa.tile([P, 32], F32, tag="alm", name="alm2")
                    nc.vector.tensor_mul(m2[:], bepm[:], selg[:, mb, :])
                    nc.vector.tensor_reduce(al2[:, mb:mb + 1], m2[:], axis=AX,
                                            op=ADD)
                nc.vector.tensor_scalar_add(al1[:], al1[:], gcol[:, 1:2])
                nc.vector.tensor_scalar_add(al2[:], al2[:], gcol[:, 1:2])

            # software-pipelined: stage A of super-tile s runs ahead of
            # stage B of super-tile s-1 so the PE never waits on adj DMA
            for s in range(NST):
                if s < 4:
                    stage_a(s)
                scale_group(s)
                if s == 3:
                    compute_alphas()
                if s >= 1:
                    stage_b(s - 1)
            stage_b(NST - 1)

    nc.compile()
    return nc


_NC_CACHE = None


def _get_program():
    global _NC_CACHE
    if _NC_CACHE is None:
        _NC_CACHE = build_program()
    return _NC_CACHE


def _to_bf16(a):
    return np.ascontiguousarray(np.asarray(a, np.float32)).astype(
        ml_dtypes.bfloat16)


def make_in_maps(x, weight, att_vec, adj):
    x = np.asarray(x, dtype=np.float32)
    weight = np.ascontiguousarray(np.asarray(weight, dtype=np.float32))
    att_vec = np.asarray(att_vec, dtype=np.float32)
    adj = np.asarray(adj, dtype=np.int32)

    xt = _to_bf16(x.T)                             # [512, 8192] bf16
    attb = np.broadcast_to(att_vec[:, 0][None, :], (P, 2 * FOUT)).copy()
    adjT = adj.T                                   # [8192, 8192] int32 view

    emat = np.zeros((P, 4, P), np.float32)
    for i in range(P):
        if i < 64:
            emat[2 * i, 0, i] = 1
            emat[2 * i + 1, 2, i] = 1
        else:
            emat[2 * i - 128, 1, i] = 1
            emat[2 * i - 127, 3, i] = 1
    emat = _to_bf16(emat)

    in_maps = []
    for c in range(NCORES):
        g = 1.0 if c < 4 else 0.0
        gcol = np.empty((P, 2), np.float32)
        gcol[:, 0] = g
        gcol[:, 1] = 1.0 - g
        selg = np.zeros((P, MB, 32), np.float32)
        for mb in range(MB):
            selg[:, mb, (8 * c + mb) % 32] = g
        in_maps.append({
            "xt": xt,
            "w": weight,
            "attb": attb,
            # (s, t, p, r) -> (s, p, t, r): one contiguous 32KB run per
            # SBUF partition per super-tile
            "adjt": np.ascontiguousarray(
                adjT[:, c * RPC:(c + 1) * RPC]
                .reshape(NST, NST, P, RPC)
                .transpose(0, 2, 1, 3)
                .reshape(NST * P, NST * RPC)),
            "gcol": gcol,
            "selg": selg,
            "emat": emat,
        })
    return in_maps


def kernel(x, weight, att_vec, adj, _trace=False, _trace_kwargs=None):
    nc = _get_program()
    in_maps = make_in_maps(x, weight, att_vec, adj)
    r = run_bass_kernel_spmd(nc, in_maps, core_ids=list(range(NCORES)),
                             trace=_trace, **(_trace_kwargs or {}))
    y = np.concatenate([r.results[c]["y"] for c in range(NCORES)], axis=0)
    kernel.last_results = r
    return y.astype(np.float32)


# revision 31
# speedup vs baseline: 1.2217x; 1.0233x over previous
"""GAT layer (nn_GAT_layer_67619965108552) as a Trainium2 Bass/Tile SPMD kernel.

Structure exploited (validated vs reference to 5.2e-3 in bf16):
  With n=8192, the buggy-but-faithful pair indexing collapses:
    rows i < 4096:  scores[i, j] = u[2i + (j >= 4096)],  u = x @ (W@a1 + W@a2)
    rows i >= 4096: scores[i, j] = tt[j mod 4096],  tt[q] = s1[2q] + s2[2q+1]
  After leaky_relu + adj masking + softmax, attn @ out reduces to masked
  row-sum matmuls against f-scaled out:
    ysb[h][mb] = sum_{kc in half h} adjT(mb,kc)^T @ (f * [out | 1])[kc]
    res = sigmoid((al1*ysbL + al2*ysbR)[:, :256] / (...)[:, 256])
  Top-half cores: f = 1, al1/al2 = exp(lrelu(u at even/odd rows)); bottom-half
  cores: f = exp(lrelu(tt)), al = 1. One instruction stream for all cores;
  divergence is data-driven (g flag / select masks).

Single fused pipeline in 8 super-tiles: x^T (bf16) streams on the HWDGE queue
while the per-core transposed adj slice (int32 -> bf16 cast) streams on the
SWDGE queue in 4MB column-group DMAs with 4KB-contiguous runs — no on-chip
transposes at all; u/s1/s2 come free as 3 extra matmul columns; per-pair
score vectors are extracted with constant 0/1 parity-pick matmuls; f-scaling
is per-chunk so nothing serializes globally.

Sharding: rows of adj / output across 8 cores (1024 each); x/weight/att_vec
replicated (each core computes the full out = x@W as the rhs of its matmuls).
"""
import ml_dtypes
import numpy as np
from contextlib import ExitStack

import concourse.bass as bass
import concourse.tile as tile
from concourse import bacc, mybir
from concourse.bass_utils import run_bass_kernel_spmd

F32 = mybir.dt.float32
BF16 = mybir.dt.bfloat16
I32 = mybir.dt.int32

N = 8192           # nodes
FIN = 512          # input features
FOUT = 256         # output features
P = 128
NB = N // P        # 64 row-blocks of out
NCORES = 8
RPC = N // NCORES  # 1024 rows per core
MB = RPC // P      # 8 output row-blocks per core
NST = 8            # super-tiles (adj column groups of 1024)


def build_program():
    nc = bacc.Bacc("TRN2", target_bir_lowering=False, debug=False,
                   num_devices=NCORES)

    xt_d = nc.dram_tensor("xt", [FIN, N], BF16, kind="ExternalInput")
    w_d = nc.dram_tensor("w", [FIN, FOUT], F32, kind="ExternalInput")
    attb_d = nc.dram_tensor("attb", [P, 2 * FOUT], F32, kind="ExternalInput")
    # per-core transposed adj slice, partition-contiguous layout:
    # adjt[s*128 + p, t*1024 + r] = adj[c*RPC + r, s*1024 + t*128 + p]
    adjt_d = nc.dram_tensor("adjt", [NST * P, NST * RPC], I32,
                            kind="ExternalInput")
    # gcol[:, 0] = g (1.0 top-half cores, 0.0 bottom), gcol[:, 1] = 1-g
    g_d = nc.dram_tensor("gcol", [P, 2], F32, kind="ExternalInput")
    # selg[p, mb, m] = g * (m == 8c + mb) : per-core pair select for alphas
    selg_d = nc.dram_tensor("selg", [P, MB, 32], F32, kind="ExternalInput")
    # parity-pick matrices [E0 | E1 | Eo0 | Eo1]
    emat_d = nc.dram_tensor("emat", [P, 4, P], BF16, kind="ExternalInput")
    y_d = nc.dram_tensor("y", [RPC, FOUT], F32, kind="ExternalOutput")

    Exp = mybir.ActivationFunctionType.Exp
    Sigmoid = mybir.ActivationFunctionType.Sigmoid
    AX = mybir.AxisListType.X
    ADD = mybir.AluOpType.add
    MULT = mybir.AluOpType.mult

    with tile.TileContext(nc) as tc, ExitStack() as ctx:
        constp = ctx.enter_context(tc.tile_pool(name="const", bufs=1))

        wtile = constp.tile([P, 4, FOUT], F32)
        attb = constp.tile([P, 2 * FOUT], F32)

        # persistent tensors
        wrhs = constp.tile([P, 4, FOUT + 3], BF16)   # [W | wu | w1 | w2]
        outb = constp.tile([P, NB, FOUT + 1], BF16)  # f*[out | 1] chunks
        aepm = constp.tile([P, 32], F32)
        bepm = constp.tile([P, 32], F32)
        fpm = constp.tile([P, 32], F32)
        al1 = constp.tile([P, MB], F32)
        al2 = constp.tile([P, MB], F32)
        ysb = [constp.tile([P, MB, FOUT + 1], F32, name=f"ysb{h}")
               for h in range(2)]
        emat = constp.tile([P, 4, P], BF16)
        gcol = constp.tile([P, 2], F32)
        selg = constp.tile([P, MB, 32], F32)

        with tc.tile_pool(name="sa", bufs=3) as sa, \
             tc.tile_pool(name="xtp", bufs=4) as xtp, \
             tc.tile_pool(name="atgp", bufs=4) as atgp, \
             tc.tile_pool(name="ps_po", bufs=2, space="PSUM") as ps_po, \
             tc.tile_pool(name="ps_y", bufs=2, space="PSUM") as ps_y, \
             tc.tile_pool(name="ps_u", bufs=2, space="PSUM") as ps_u:

            # Single DMA queue (SWDGE) carries the big streams in explicit
            # priority order — the HWDGE/SWDGE queues share the same 16 SDMA
            # engines, so two busy queues just fight. Small consts ride the
            # otherwise-idle sync ring during the stream ramp.
            nc.sync.dma_start(wtile[:],
                              w_d.ap().rearrange("(c p) f -> p c f", p=P))
            nc.sync.dma_start(attb[:], attb_d.ap())
            nc.sync.dma_start(emat[:], emat_d.ap())
            nc.sync.dma_start(gcol[:], g_d.ap())
            nc.sync.dma_start(selg[:], selg_d.ap())

            xts_list = []
            atg_list = []

            def emit_xts(s2, half):
                # xt host layout: row s*128+p, col h*4096 + c*1024 + r
                if half == 0:
                    xts_list.append(xtp.tile([P, 4, 16 * P], BF16, tag="xts",
                                             name="xts"))
                xts = xts_list[s2]
                nc.gpsimd.dma_start(
                    xts[:, :, half * 1024:(half + 1) * 1024],
                    xt_d.ap()[s2 * P:(s2 + 1) * P,
                              half * 4096:(half + 1) * 4096].rearrange(
                        "p (c r) -> p c r", c=4))

            def emit_atg(s2, lo_t, hi_t):
                if lo_t == 0:
                    atg_list.append(atgp.tile([P, NST, RPC], BF16, tag="atg",
                                              name="atg"))
                atg = atg_list[s2]
                nc.gpsimd.dma_start(
                    atg[:, lo_t:hi_t, :],
                    adjt_d.ap()[s2 * P:(s2 + 1) * P,
                                lo_t * 1024:hi_t * 1024].rearrange(
                        "p (t r) -> p t r", t=hi_t - lo_t))

            emit_xts(0, 0)
            emit_xts(0, 1)
            emit_atg(0, 0, 4)
            emit_xts(1, 0)
            emit_xts(1, 1)
            emit_atg(0, 4, 8)
            emit_xts(2, 0)
            emit_xts(2, 1)
            emit_atg(1, 0, 4)
            emit_atg(1, 4, 8)
            emit_xts(3, 0)
            emit_xts(3, 1)
            for s2 in range(2, NST - 1):
                emit_atg(s2, 0, 4)
                emit_atg(s2, 4, 8)
            # last group in finer pieces so its stage-B chains start earlier
            for lo in range(0, NST, 2):
                emit_atg(NST - 1, lo, lo + 2)

            # build wrhs = [W | W@(a1+a2) | W@a1 | W@a2] per k-chunk, bf16
            wamf = sa.tile([P, 4, 3], F32, tag="wamf", name="wamf")
            for c in range(4):
                t1 = sa.tile([P, FOUT], F32, tag="wa_tmp", name="wa1")
                nc.vector.tensor_mul(t1[:], wtile[:, c, :], attb[:, :FOUT])
                nc.vector.tensor_reduce(wamf[:, c, 1:2], t1[:], axis=AX, op=ADD)
                t2 = sa.tile([P, FOUT], F32, tag="wa_tmp", name="wa2")
                nc.vector.tensor_mul(t2[:], wtile[:, c, :], attb[:, FOUT:])
                nc.vector.tensor_reduce(wamf[:, c, 2:3], t2[:], axis=AX, op=ADD)
                nc.vector.tensor_add(wamf[:, c, 0:1], wamf[:, c, 1:2],
                                     wamf[:, c, 2:3])
                nc.vector.tensor_copy(wrhs[:, c, :FOUT], wtile[:, c, :])
                nc.vector.tensor_copy(wrhs[:, c, FOUT:], wamf[:, c, :])

            def stage_a(g):
                xts = xts_list[g]
                usbE = sa.tile([P, 24], BF16, tag="usbE", name="usbE")
                usbO = sa.tile([P, 24], BF16, tag="usbO", name="usbO")
                for pair in range(8):
                    for half in range(2):
                        lb = 2 * pair + half      # local block 0..15
                        b = 16 * g + lb           # global block
                        po = ps_po.tile([P, FOUT + 3], F32, tag="po",
                                        name="po")
                        for c in range(4):
                            nc.tensor.matmul(
                                po[:], xts[:, c, lb * P:(lb + 1) * P],
                                wrhs[:, c, :],
                                start=(c == 0), stop=(c == 3))
                        nc.vector.tensor_copy(outb[:, b, :FOUT],
                                              po[:, :FOUT])
                        dst = usbE if half == 0 else usbO
                        nc.scalar.copy(dst[:, pair * 3:(pair + 1) * 3],
                                       po[:, FOUT:FOUT + 3])
                # parity-pick matmuls: even picks cols 0:24, odd 24:48
                pv = ps_u.tile([P, 48], F32, tag="pv", name="pv")
                nc.tensor.matmul(pv[:, 0:24], emat[:, 0, :], usbE[:],
                                 start=True, stop=False)
                nc.tensor.matmul(pv[:, 0:24], emat[:, 1, :], usbO[:],
                                 start=False, stop=True)
                nc.tensor.matmul(pv[:, 24:48], emat[:, 2, :], usbE[:],
                                 start=True, stop=False)
                nc.tensor.matmul(pv[:, 24:48], emat[:, 3, :], usbO[:],
                                 start=False, stop=True)
                pvs = sa.tile([P, 48], F32, tag="pvs", name="pvs")
                nc.vector.tensor_copy(pvs[:], pv[:])
                # abv = [ae_pre(8) | be_pre(8) | vv_pre(8)]
                abv = sa.tile([P, 24], F32, tag="abv", name="abv")
                nc.vector.tensor_copy(abv[:, 0:8], pvs[:, 0:24:3])
                nc.vector.tensor_copy(abv[:, 8:16], pvs[:, 24:48:3])
                nc.vector.tensor_add(abv[:, 16:24], pvs[:, 1:24:3],
                                     pvs[:, 26:48:3])
                tmp = sa.tile([P, 24], F32, tag="abt", name="abt")
                nc.vector.tensor_scalar_mul(tmp[:], abv[:], 0.01)
                nc.vector.tensor_max(abv[:], abv[:], tmp[:])
                abve = sa.tile([P, 24], F32, tag="abve", name="abve")
                nc.scalar.activation(abve[:], abv[:], Exp)
                sl = slice(8 * g, 8 * g + 8)
                nc.vector.tensor_copy(aepm[:, sl], abve[:, 0:8])
                nc.vector.tensor_copy(bepm[:, sl], abve[:, 8:16])
                nc.vector.tensor_scalar(fpm[:, sl], abve[:, 16:24],
                                        gcol[:, 1:2], gcol[:, 0:1],
                                        op0=MULT, op1=ADD)

            def scale_group(g):
                klo = NST * g
                m0 = klo % 32
                for t in range(NST):
                    kc = klo + t
                    m = kc % 32
                    nc.scalar.activation(outb[:, kc, :FOUT],
                                         outb[:, kc, :FOUT],
                                         mybir.ActivationFunctionType.Copy,
                                         scale=fpm[:, m:m + 1])
                nc.scalar.copy(outb[:, klo:klo + NST, FOUT:FOUT + 1],
                               fpm[:, m0:m0 + NST])

            def stage_b(g):
                klo = NST * g
                atg = atg_list[g]
                h = 0 if g < 4 else 1
                for mb in range(MB):
                    yp = ps_y.tile([P, FOUT + 1], F32, tag="yp", name="yp")
                    for t in range(NST):
                        kc = klo + t
                        nc.tensor.matmul(yp[:],
                                         atg[:, t, mb * P:(mb + 1) * P],
                                         outb[:, kc, :],
                                         start=(t == 0), stop=(t == NST - 1))
                    if g % 4 == 0:
                        nc.vector.tensor_copy(ysb[h][:, mb, :], yp[:])
                    elif g < NST - 1:
                        nc.vector.tensor_add(ysb[h][:, mb, :],
                                             ysb[h][:, mb, :], yp[:])
                    else:
                        # final: z = zpre + al2*yp; res = sigmoid(z/:z[256])
                        z2 = sa.tile([P, FOUT + 1], F32, tag="z2", name="z2")
                        nc.vector.tensor_scalar_mul(z2[:], yp[:],
                                                    al2[:, mb:mb + 1])
                        nc.vector.tensor_add(z2[:], z2[:], ysb[1][:, mb, :])
                        rec = sa.tile([P, 1], F32, tag="rec", name="rec")
                        nc.vector.reciprocal(rec[:], z2[:, FOUT:FOUT + 1])
                        resg = sa.tile([P, FOUT], F32, tag="resg",
                                       name="resg")
                        nc.scalar.activation(resg[:], z2[:, :FOUT], Sigmoid,
                                             scale=rec[:])
                        nc.sync.dma_start(y_d.ap()[mb * P:(mb + 1) * P, :],
                                          resg[:])
                if g == NST - 2:
                    # fold al1*ysbL + al2*ysbR(partial) into ysb[1] ahead of
                    # the final group so the tail is short
                    for mb in range(MB):
                        z1 = sa.tile([P, FOUT + 1], F32, tag="z1", name="z1")
                        nc.vector.tensor_scalar_mul(z1[:], ysb[0][:, mb, :],
                                                    al1[:, mb:mb + 1])
                        nc.vector.tensor_scalar_mul(ysb[1][:, mb, :],
                                                    ysb[1][:, mb, :],
                                                    al2[:, mb:mb + 1])
                        nc.vector.tensor_add(ysb[1][:, mb, :],
                                             ysb[1][:, mb, :], z1[:])

            def compute_alphas():
                for mb in range(MB):
                    m1 = sa.tile([P, 32], F32, tag="alm", name="alm1")
                    nc.vector.tensor_mul(m1[:], aepm[:], selg[:, mb, :])
                    nc.vector.tensor_reduce(al1[:, mb:mb + 1], m1[:], axis=AX,
                                            op=ADD)
                    m2 = sa.tile([P, 32], F32, tag="alm", name="alm2")
                    nc.vector.tensor_mul(m2[:], bepm[:], selg[:, mb, :])
                    nc.vector.tensor_reduce(al2[:, mb:mb + 1], m2[:], axis=AX,
                                            op=ADD)
                nc.vector.tensor_scalar_add(al1[:], al1[:], gcol[:, 1:2])
                nc.vector.tensor_scalar_add(al2[:], al2[:], gcol[:, 1:2])

            # software-pipelined: stage A of super-tile s runs ahead of
            # stage B of super-tile s-1 so the PE never waits on adj DMA
            for s in range(NST):
                if s < 4:
                    stage_a(s)
                scale_group(s)
                if s == 3:
                    compute_alphas()
                if s >= 1:
                    stage_b(s - 1)
            stage_b(NST - 1)

    nc.compile()
    return nc


_NC_CACHE = None


def _get_program():
    global _NC_CACHE
    if _NC_CACHE is None:
        _NC_CACHE = build_program()
    return _NC_CACHE


def _to_bf16(a):
    return np.ascontiguousarray(np.asarray(a, np.float32)).astype(
        ml_dtypes.bfloat16)


def make_in_maps(x, weight, att_vec, adj):
    x = np.asarray(x, dtype=np.float32)
    weight = np.ascontiguousarray(np.asarray(weight, dtype=np.float32))
    att_vec = np.asarray(att_vec, dtype=np.float32)
    adj = np.asarray(adj, dtype=np.int32)

    # x^T in bf16, permuted so each SBUF partition reads contiguous runs:
    # row s*128+p, col h*4096 + c*1024 + r  <-  xT[c*128+p, s*2048+h*1024+r]
    xt = _to_bf16(np.ascontiguousarray(
        x.T.reshape(4, P, 4, 2, 1024).transpose(2, 1, 3, 0, 4)
        .reshape(FIN, N)))
    attb = np.broadcast_to(att_vec[:, 0][None, :], (P, 2 * FOUT)).copy()
    adjT = adj.T                                   # [8192, 8192] int32 view

    emat = np.zeros((P, 4, P), np.float32)
    for i in range(P):
        if i < 64:
            emat[2 * i, 0, i] = 1
            emat[2 * i + 1, 2, i] = 1
        else:
            emat[2 * i - 128, 1, i] = 1
            emat[2 * i - 127, 3, i] = 1
    emat = _to_bf16(emat)

    in_maps = []
    for c in range(NCORES):
        g = 1.0 if c < 4 else 0.0
        gcol = np.empty((P, 2), np.float32)
        gcol[:, 0] = g
        gcol[:, 1] = 1.0 - g
        selg = np.zeros((P, MB, 32), np.float32)
        for mb in range(MB):
            selg[:, mb, (8 * c + mb) % 32] = g
        in_maps.append({
            "xt": xt,
            "w": weight,
            "attb": attb,
            # (s, t, p, r) -> (s, p, t, r): one contiguous 32KB run per
            # SBUF partition per super-tile
            "adjt": np.ascontiguousarray(
                adjT[:, c * RPC:(c + 1) * RPC]
                .reshape(NST, NST, P, RPC)
                .transpose(0, 2, 1, 3)
                .reshape(NST * P, NST * RPC)),
            "gcol": gcol,
            "selg": selg,
            "emat": emat,
        })
    return in_maps


def kernel(x, weight, att_vec, adj, _trace=False, _trace_kwargs=None):
    nc = _get_program()
    in_maps = make_in_maps(x, weight, att_vec, adj)
    r = run_bass_kernel_spmd(nc, in_maps, core_ids=list(range(NCORES)),
                             trace=_trace, **(_trace_kwargs or {}))
    y = np.concatenate([r.results[c]["y"] for c in range(NCORES)], axis=0)
    kernel.last_results = r
    return y.astype(np.float32)
